# revision 13
# baseline (speedup 1.0000x reference)
"""3-layer GCN encoder (nn_GCNEncoder) on 8 Trainium2 NeuronCores.

Strategy (graph/data parallel, 1D node sharding):
  - Node shard c = rows [c*NPC, (c+1)*NPC).  Core c owns all edges whose
    *destination* lies in its shard (plus that shard's self-loops).
  - GCN norm is factorized:  out = dinv * (A^T (dinv * (h W))) + b: dinv is
    applied once per node before the AllGather and once at PSUM evacuation.
  - Per layer on each core:
      1. transform:  u = dinv * (h @ W)   (h feature-major in SBUF, bf16 for
         layers 1-2, fp32 for layer 3)
      2. AllGather u  ->  u_full [N, F] in local HBM (the gather table)
      3. aggregation: edges sorted by (superblock, src-group, dst-block,
         dst); chunks of 128 edges are cut at *run* granularity (a run =
         one (superblock, src-group) segment), so a chunk may straddle a
         dst-block boundary; dma_gather pulls the 256B source rows; a 0/1
         one-hot matrix S (DVE is_equal against an iota constant) scatters
         each (chunk, dst-block) slot into that block's PSUM accumulator:
         psum[F, dst] += msg^T @ S.  Straddling chunks emit one matmul per
         touched block; dstl=-1 marks edges outside the slot's block.
      4. evacuation: h_next = relu(dinv * psum + b)
  - The SPMD instruction stream is shared by all 8 cores: chunks per run =
    max over cores; slot list = union of blocks each chunk touches on any
    core; pad slots contribute nothing (dstl=-1 -> zero one-hot columns).

kernel() takes the full unsharded inputs and returns the full output.
"""

import os
import sys

import numpy as np

sys.path.insert(0, "/opt/trn_rl_repo")

P = 128
GMAX = 8           # chunks per dma_gather call; >8 wedges the NC on this stack
USE_QUEUES = False  # multi-queue SWDGE wedged the NC in bench


class Cfg:
    def __init__(self, n_nodes, n_cores, d_in, d_hid, d_out,
                 sb_blocks=8, sbatch=16, n_groups=4):
        assert n_nodes % n_cores == 0
        self.n_nodes = n_nodes
        self.n_cores = n_cores
        self.d_in, self.d_hid, self.d_out = d_in, d_hid, d_out
        self.npc = n_nodes // n_cores              # nodes per core
        self.nblk = -(-self.npc // P)              # dst blocks per core
        self.npcp = self.nblk * P                  # padded nodes per core
        self.sb_blocks = sb_blocks                 # dst blocks per superblock
        self.nsb = -(-self.nblk // sb_blocks)
        self.n_groups = n_groups                   # src-range groups
        assert n_cores % n_groups == 0
        self.cpg = n_cores // n_groups             # cores per src group
        self.grp = self.npcp * self.cpg            # padded rows per src group
        assert self.grp <= 32767, "src group must fit int16 gather indices"
        self.sbatch = sbatch                       # S-slots per one-hot build
        self.gmax = GMAX


def _host_prep(edge_index, cfg):
    """Shard edges, build the shared slot schedule and per-core streams."""
    n, ncores, npc = cfg.n_nodes, cfg.n_cores, cfg.npc
    ei = np.asarray(edge_index)
    src = ei[0]
    dst = ei[1]
    # self-loops are applied as an identity matmul per dst block on device,
    # but they count toward the degree
    deg = (np.bincount(dst, minlength=n) + 1).astype(np.float64)
    dinv = (1.0 / np.sqrt(deg)).astype(np.float32)

    core = dst // npc
    nsb, ng = cfg.nsb, cfg.n_groups
    per_core_raw = []
    runcnt = np.zeros((ncores, nsb, ng), dtype=np.int64)
    bounds = []
    for c in range(ncores):
        m = core == c
        s = src[m]
        d = (dst[m] - c * npc).astype(np.int64)
        blk = d // P
        sc = s // npc                               # source core
        grp = sc // cfg.cpg                         # src group
        sb = blk // cfg.sb_blocks
        order = np.lexsort((d, blk, grp, sb))
        s, d, blk, grp, sb = s[order], d[order], blk[order], grp[order], sb[order]
        loc = (sc[order] % cfg.cpg) * cfg.npcp + (s % npc)
        key = sb * ng + grp                          # ascending
        bnd = np.searchsorted(key, np.arange(nsb * ng + 1))
        bounds.append(bnd)
        np.add.at(runcnt[c], (sb, grp), 1)
        per_core_raw.append((loc, d, blk))

    nch_run = (-(-runcnt // P)).max(axis=0)          # [nsb, ng] shared chunks
    totch = int(nch_run.sum())
    tot_slots = totch * P                            # gather idx slots

    # shared slot schedule: per run, per chunk k, the union over cores of dst
    # blocks present in chunk k.  Slots are ordered (sb, g, k, b) so each
    # run's slots occupy a contiguous dstl column range.
    run_slots = {}                                   # (sb,g) -> [(k, b, stop)]
    slot_of = {}
    nslots = 0
    last_slot_of_block = {}
    has_slots = set()
    for sb in range(nsb):
        for g in range(ng):
            rch = int(nch_run[sb, g])
            if rch == 0:
                continue
            bk = [set() for _ in range(rch)]
            for c in range(ncores):
                lo, hi = bounds[c][sb * ng + g], bounds[c][sb * ng + g + 1]
                blkseg = per_core_raw[c][2][lo:hi]
                cnt = hi - lo
                for k in range(-(-cnt // P)):
                    seg = blkseg[k * P:min((k + 1) * P, cnt)]
                    bk[k].update(np.unique(seg).tolist())
            sl = []
            for k in range(rch):
                for b in sorted(bk[k]):
                    slot_of[(sb, g, k, b)] = nslots
                    last_slot_of_block[b] = nslots
                    has_slots.add(b)
                    sl.append([k, b, False])
                    nslots += 1
            run_slots[(sb, g)] = sl
    for (sb, g), sl in run_slots.items():
        for t in sl:
            k, b, _ = t
            if slot_of[(sb, g, k, b)] == last_slot_of_block[b]:
                t[2] = True

    # per-core streams
    per_core = []
    for c in range(ncores):
        loc, d, blk = per_core_raw[c]
        idx_all = np.zeros(tot_slots, dtype=np.int16)
        dl_all = np.full((nslots, P), -1.0, dtype=np.float32)
        pos = 0
        for sb in range(nsb):
            for g in range(ng):
                rch = int(nch_run[sb, g])
                if rch == 0:
                    continue
                lo, hi = bounds[c][sb * ng + g], bounds[c][sb * ng + g + 1]
                cnt = hi - lo
                idx_all[pos:pos + cnt] = loc[lo:hi].astype(np.int16)
                for k in range(-(-cnt // P)):
                    e0, e1 = k * P, min((k + 1) * P, cnt)
                    seg_b = blk[lo + e0:lo + e1]
                    seg_d = d[lo + e0:lo + e1]
                    for b in np.unique(seg_b):
                        si = slot_of[(sb, g, k, int(b))]
                        msk = seg_b == b
                        dl_all[si, np.nonzero(msk)[0]] = (
                            seg_d[msk] - b * P).astype(np.float32)
                pos += rch * P
        assert pos == tot_slots
        a16 = idx_all.reshape(tot_slots // 16, 16).T
        idx_wrapped = np.ascontiguousarray(np.tile(a16, (8, 1)))
        dstl = np.ascontiguousarray(dl_all.T)        # [128, nslots]
        per_core.append({"idx": idx_wrapped, "dstl": dstl})

    sched = {
        "nch_run": nch_run,
        "run_slots": run_slots,
        "nslots": nslots,
        "totch": totch,
        "tot16": tot_slots // 16,
        "maxrun": int(nch_run.max()),
        "has_slots": has_slots,
        "dinv": dinv,
    }
    return sched, per_core


def build_nc(cfg, sched, debug=False):
    from concourse import bacc, mybir

    f32 = mybir.dt.float32
    bf16 = mybir.dt.bfloat16
    i16 = mybir.dt.int16
    Alu = mybir.AluOpType
    Act = mybir.ActivationFunctionType

    npc, nblk = cfg.npc, cfg.nblk
    nslots, tot16, maxrun = sched["nslots"], sched["tot16"], sched["maxrun"]
    nch_run, run_slots = sched["nch_run"], sched["run_slots"]
    has_slots = sched["has_slots"]
    layer_dims = [(cfg.d_in, cfg.d_hid), (cfg.d_hid, cfg.d_hid),
                  (cfg.d_hid, cfg.d_out)]
    ldt = [bf16, bf16, f32]                     # gather-table dtype per layer

    nc = bacc.Bacc("TRN2", target_bir_lowering=False, debug=debug,
                   enable_asserts=False, num_devices=cfg.n_cores,
                   num_swdge_queues=2 if USE_QUEUES else 1)

    xT = nc.dram_tensor("xT", [P, cfg.npcp], bf16, kind="ExternalInput")
    Wd, Bd = [], []
    for li, (fi, fo) in enumerate(layer_dims):
        Wd.append(nc.dram_tensor(f"W{li + 1}", [fi, fo], bf16, kind="ExternalInput"))
        Bd.append(nc.dram_tensor(f"B{li + 1}", [fo, 1], f32, kind="ExternalInput"))
    dinv_col_d = nc.dram_tensor("dinv_col", [P, nblk], f32, kind="ExternalInput")
    dinvb_d = nc.dram_tensor("dinvb", [P, cfg.npcp], f32, kind="ExternalInput")
    iota_d = nc.dram_tensor("iota_t", [P, cfg.sbatch * P], f32, kind="ExternalInput")
    iota16_d = nc.dram_tensor("iota16", [P, cfg.sbatch * P], bf16, kind="ExternalInput")
    ident_d = nc.dram_tensor("ident", [P, P], f32, kind="ExternalInput")
    ident16_d = nc.dram_tensor("ident16", [P, P], bf16, kind="ExternalInput")
    idx_d = nc.dram_tensor("idxs", [P, tot16], i16, kind="ExternalInput")
    dstl_d = nc.dram_tensor("dstl", [P, nslots], f32, kind="ExternalInput")
    dstl16_d = nc.dram_tensor("dstl16", [P, nslots], bf16, kind="ExternalInput")
    outT = nc.dram_tensor("outT", [cfg.d_out, cfg.npcp], f32, kind="ExternalOutput")

    u_own, u_full = [], []
    for li, (fi, fo) in enumerate(layer_dims):
        u_own.append(nc.dram_tensor(f"u_own{li + 1}", [cfg.npcp, fo], ldt[li]))
        u_full.append(nc.dram_tensor(f"u_full{li + 1}",
                                     [cfg.n_cores * cfg.npcp, fo], ldt[li],
                                     addr_space="Shared"))

    from concourse import tile

    rg = [list(range(cfg.n_cores))]
    qctr = [0]

    def next_q():
        if not USE_QUEUES:
            return 0
        qctr[0] ^= 1
        return qctr[0]

    with tile.TileContext(nc) as tc:
        with (
            tc.tile_pool(name="const", bufs=1) as constp,
            tc.tile_pool(name="hbuf", bufs=1) as hp,
            tc.tile_pool(name="gath", bufs=3) as gp,
            tc.tile_pool(name="gidx", bufs=3) as ip,
            tc.tile_pool(name="sel", bufs=4) as sp,
            tc.tile_pool(name="dinvb", bufs=2) as dbp,
            tc.tile_pool(name="evac", bufs=3) as tp,
            tc.tile_pool(name="ustage", bufs=3) as up,
            tc.tile_pool(name="accp", bufs=cfg.sb_blocks, space="PSUM") as accp,
        ):
            from concourse import library_config
            nc.gpsimd.load_library(library_config.mlp)

            # constants
            wt, bt = [], []
            for li, (fi, fo) in enumerate(layer_dims):
                w = constp.tile([fi, fo], bf16, tag=f"w{li}")
                nc.sync.dma_start(w[:], Wd[li][:])
                wt.append(w)
                b = constp.tile([fo, 1], f32, tag=f"b{li}")
                nc.sync.dma_start(b[:], Bd[li][:])
                bt.append(b)
            dct = constp.tile([P, nblk], f32, tag="dct")
            nc.sync.dma_start(dct[:], dinv_col_d[:])
            iot = constp.tile([P, cfg.sbatch * P], f32, tag="iot")
            nc.sync.dma_start(iot[:], iota_d[:])
            iot16 = constp.tile([P, cfg.sbatch * P], bf16, tag="iot16")
            nc.sync.dma_start(iot16[:], iota16_d[:])
            idt = constp.tile([P, P], f32, tag="idt")
            nc.sync.dma_start(idt[:], ident_d[:])
            idt16 = constp.tile([P, P], bf16, tag="idt16")
            nc.sync.dma_start(idt16[:], ident16_d[:])
            dlt = constp.tile([P, nslots], f32, tag="dlt")
            nc.sync.dma_start(dlt[:], dstl_d[:])
            dlt16 = constp.tile([P, nslots], bf16, tag="dlt16")
            nc.sync.dma_start(dlt16[:], dstl16_d[:])

            h = hp.tile([P, cfg.npcp], bf16, tag="h")
            nc.sync.dma_start(h[:], xT[:])

            for li, (fi, fo) in enumerate(layer_dims):
                last_layer = li == len(layer_dims) - 1
                lt = ldt[li]
                utag = "u16" if lt == bf16 else "u32"
                # ---- transform: u = dinv * (h @ W) ----
                for b in range(nblk):
                    off = b * P
                    pt = accp.tile([P, P], f32, tag="acc",
                                    name=f"tf{li}_{b}")
                    nc.tensor.matmul(pt[:P, :fo], lhsT=h[:fi, off:off + P],
                                     rhs=wt[li][:, :fo], start=True, stop=True)
                    ut = up.tile([P, P], lt, tag=utag)
                    nc.vector.tensor_scalar_mul(ut[:P, :fo], pt[:P, :fo],
                                                dct[:P, b:b + 1])
                    nc.sync.dma_start(u_own[li][off:off + P, :], ut[:P, :fo])

                # ---- AllGather the transformed features ----
                nc.gpsimd.collective_compute(
                    "AllGather", mybir.AluOpType.bypass, replica_groups=rg,
                    ins=[u_own[li][:]], outs=[u_full[li][:]],
                )

                # ---- aggregation ----
                icol = 0
                slotbase = 0
                myiot = iot16 if lt == bf16 else iot
                mydlt = dlt16 if lt == bf16 else dlt
                myidt = idt16 if lt == bf16 else idt
                for sb in range(cfg.nsb):
                    blocks = list(range(sb * cfg.sb_blocks,
                                        min((sb + 1) * cfg.sb_blocks, nblk)))
                    # self-loop contribution opens each block's accumulation
                    sb_off = blocks[0] * P
                    nfull = len(blocks)
                    ublk = gp.tile([P, cfg.sb_blocks * P], lt,
                                   tag=f"ublk_{utag}", name=f"ublk{li}_{sb}")
                    nc.sync.dma_start(
                        ublk[:, :nfull * fo].rearrange("p (c f) -> p c f", f=fo),
                        u_own[li][sb_off:sb_off + nfull * P, :].rearrange(
                            "(c p) f -> p c f", p=P))
                    acc = {}
                    for b in blocks:
                        ci = b - blocks[0]
                        acc[b] = accp.tile([P, P], f32, tag="acc",
                                           name=f"acc{li}_{b}")
                        nc.tensor.matmul(
                            acc[b][:fo, :],
                            lhsT=ublk[:P, ci * fo:ci * fo + fo],
                            rhs=myidt[:P, :],
                            start=True,
                            stop=b not in has_slots,
                        )
                    for g in range(cfg.n_groups):
                        rch = int(nch_run[sb, g])
                        if rch == 0:
                            continue
                        sl = run_slots[(sb, g)]
                        l16 = rch * 8
                        it = ip.tile([P, maxrun * 8], i16, tag="it")
                        nc.sync.dma_start(it[:, :l16], idx_d[:, icol:icol + l16])
                        wbase = 0
                        for c0 in range(0, rch, cfg.gmax):
                            gn = min(cfg.gmax, rch - c0)
                            gt = gp.tile([P, cfg.gmax * fo], lt,
                                         tag=f"gt_{utag}")
                            nc.gpsimd.dma_gather(
                                out_ap=gt[:, :gn * fo].rearrange(
                                    "p (c e) -> p c e", e=fo),
                                in_ap=u_full[li][g * cfg.grp:(g + 1) * cfg.grp, :],
                                idxs_ap=it[:, c0 * 8:(c0 + gn) * 8],
                                num_idxs=gn * P,
                                num_idxs_reg=gn * P,
                                elem_size=fo,
                                queue_num=next_q(),
                            )
                            # slots whose chunk lies in this gather window
                            wlo = wbase
                            while wbase < len(sl) and sl[wbase][0] < c0 + gn:
                                wbase += 1
                            wsl = sl[wlo:wbase]
                            for s0 in range(0, len(wsl), cfg.sbatch):
                                batch = wsl[s0:s0 + cfg.sbatch]
                                kk = len(batch)
                                st = sp.tile([P, cfg.sbatch * P], lt,
                                             tag=f"st_{utag}")
                                cbase = slotbase + wlo + s0
                                in1 = mydlt[:, cbase:cbase + kk].rearrange(
                                    "p (c o) -> p c o", o=1).to_broadcast(
                                        [P, kk, P])
                                nc.vector.tensor_tensor(
                                    out=st[:, :kk * P].rearrange(
                                        "p (c e) -> p c e", e=P),
                                    in0=myiot[:, :kk * P].rearrange(
                                        "p (c e) -> p c e", e=P),
                                    in1=in1,
                                    op=Alu.is_equal,
                                )
                                for j, (k, b, stop) in enumerate(batch):
                                    nc.tensor.matmul(
                                        acc[b][:fo, :],
                                        lhsT=gt[:, (k - c0) * fo:(k - c0 + 1) * fo],
                                        rhs=st[:, j * P:(j + 1) * P],
                                        start=False, stop=stop,
                                    )
                        icol += l16
                        slotbase += len(sl)

                    # ---- evacuate superblock ----
                    sb_w = (blocks[-1] + 1) * P - sb_off
                    dbt = dbp.tile([P, cfg.sb_blocks * P], f32, tag="dbt")
                    nc.sync.dma_start(dbt[:, :sb_w],
                                      dinvb_d[:, sb_off:sb_off + sb_w])
                    for b in blocks:
                        off = b * P
                        tt = tp.tile([P, P], f32, tag="tt")
                        nc.vector.tensor_tensor(
                            tt[:fo, :P], in0=acc[b][:fo, :P],
                            in1=dbt[:fo, off - sb_off:off - sb_off + P],
                            op=Alu.mult)
                        if not last_layer:
                            nc.scalar.activation(h[:fo, off:off + P],
                                                 tt[:fo, :P], Act.Relu,
                                                 bias=bt[li][:, :1])
                        else:
                            ot = up.tile([P, P], f32, tag="u32")
                            nc.vector.tensor_scalar_add(ot[:fo, :P],
                                                        tt[:fo, :P],
                                                        bt[li][:, :1])
                            nc.sync.dma_start(outT[:, off:off + P],
                                              ot[:fo, :P])
    nc.finalize()
    return nc


def make_in_maps(x, W1, b1, W2, b2, W3, b3, cfg, sched, per_core):
    import ml_dtypes
    bf = ml_dtypes.bfloat16
    x = np.ascontiguousarray(np.asarray(x, dtype=np.float32))
    dinv = sched["dinv"]
    npc, nblk = cfg.npc, cfg.nblk
    iota = np.tile(np.arange(P, dtype=np.float32), (P, cfg.sbatch))
    common = {
        "W1": np.ascontiguousarray(np.asarray(W1, np.float32)).astype(bf),
        "W2": np.ascontiguousarray(np.asarray(W2, np.float32)).astype(bf),
        "W3": np.ascontiguousarray(np.asarray(W3, np.float32)).astype(bf),
        "B1": np.asarray(b1, np.float32).reshape(-1, 1).copy(),
        "B2": np.asarray(b2, np.float32).reshape(-1, 1).copy(),
        "B3": np.asarray(b3, np.float32).reshape(-1, 1).copy(),
        "iota_t": np.ascontiguousarray(iota),
        "iota16": np.ascontiguousarray(iota).astype(bf),
        "ident": np.eye(P, dtype=np.float32),
        "ident16": np.eye(P, dtype=np.float32).astype(bf),
    }
    in_maps = []
    for c in range(cfg.n_cores):
        dv_pad = np.zeros(cfg.npcp, np.float32)
        dv_pad[:npc] = dinv[c * npc:(c + 1) * npc]
        xT = np.zeros((P, cfg.npcp), np.float32)
        xT[:, :npc] = x[c * npc:(c + 1) * npc].T
        m = dict(common)
        m["xT"] = xT.astype(bf)
        m["dinv_col"] = np.ascontiguousarray(dv_pad.reshape(nblk, P).T)
        m["dinvb"] = np.ascontiguousarray(np.broadcast_to(dv_pad, (P, cfg.npcp)))
        m["idxs"] = per_core[c]["idx"]
        m["dstl"] = per_core[c]["dstl"]
        m["dstl16"] = per_core[c]["dstl"].astype(bf)
        in_maps.append(m)
    return in_maps


def assemble(results, cfg):
    out = np.empty((cfg.n_nodes, cfg.d_out), dtype=np.float32)
    for c in range(cfg.n_cores):
        out[c * cfg.npc:(c + 1) * cfg.npc, :] = results[c]["outT"].T[:cfg.npc]
    return out


def full_cfg():
    return Cfg(n_nodes=100000, n_cores=8, d_in=128, d_hid=128, d_out=64)


_CACHE = {}


def _install_ntff_hook():
    """Register the axon NTFF profiling hook if the image's antenv lacks it."""
    try:
        import types

        import antenv
        try:
            from antenv.axon_hooks import get_axon_ntff_profile_hook  # noqa: F401
            return
        except ImportError:
            pass
        from trn_agent_boot.trn_boot import _ntff_profile_via_ctypes
        mod = types.ModuleType("antenv.axon_hooks")
        state = {"hook": None}
        mod.set_axon_ntff_profile_hook = lambda h: state.__setitem__("hook", h)
        mod.get_axon_ntff_profile_hook = lambda: state["hook"]
        sys.modules["antenv.axon_hooks"] = mod
        antenv.axon_hooks = mod
        mod.set_axon_ntff_profile_hook(
            _ntff_profile_via_ctypes("/opt/axon/libaxon_pjrt.so"))
    except Exception as e:  # degrade to no tracing
        print(f"ntff hook install failed: {e}")


def kernel(x, edge_index, W1, b1, W2, b2, W3, b3):
    from concourse.bass_utils import run_bass_kernel_spmd

    cfg = full_cfg()
    sched, per_core = _host_prep(np.asarray(edge_index), cfg)
    key = "full"
    if key not in _CACHE:
        _CACHE[key] = build_nc(cfg, sched)
    nc = _CACHE[key]
    in_maps = make_in_maps(x, W1, b1, W2, b2, W3, b3, cfg, sched, per_core)
    trace = bool(int(os.environ.get("GCN_TRACE", "0")))
    if trace:
        _install_ntff_hook()
    res = run_bass_kernel_spmd(nc, in_maps, core_ids=list(range(cfg.n_cores)),
                               trace=trace)
    if res.exec_time_ns is not None:
        print(f"HW exec time: {res.exec_time_ns} ns")
    return assemble(res.results, cfg)
